# revision 59
# baseline (speedup 1.0000x reference)
"""Trainium2 Bass kernel for nn_Atten_Block (non-local attention block).

Reference computation per batch element b (C=256, C4=64, H=W=64, N=4096):
    theta = W1 @ x + b1          [C4, N]
    phi   = W2 @ x + b2          [C4, N]
    g     = W3 @ x + b3          [C4, N]
    S     = theta^T @ phi        [N, N]
    A     = softmax(S, axis=-1)
    attn_g[c,i] = sum_j g[c,j] A[i,j]
    y     = x + W4 @ attn_g + b4

Sharding: data-parallel over batch B=8 across the 8 NeuronCores (one batch
element per core).

Per-core algorithm v2 — keep the PE in sustained 2-stream mode:
  - The PE streams ONE rhs column per cycle per resident tile, and can run
    two tiles CONCURRENTLY when they sit on different row-groups (rows
    0-63 / 64-127) and drain to different PSUM banks (measured 4.6 Gcol/s
    vs 2.33 for serial full-row chains).  So EVERY hot matmul is emitted as
    a half-array tile:
      * S^T tiles (K=64) row-pack even jb on rows 0-63, odd on 64-127
        (theta/phi duplicated across partition halves), as before.
      * PV is split into j-half chains: for each 128-j block, the lower
        64 j's accumulate into psum bank pvA via a (0,0) tile and the upper
        64 j's into pvB via a (64,0) tile.  The two chains (plus their
        appended ones-column -> partial softmax denominators) are merged
        after the i-tile completes: mg = pvA + pvB via one ACT Identity
        evac + one DVE add, into SBUF.
      * Emission interleaves S and PV halves on opposite row groups
        [S_even | pvB(j-hi)] [S_odd | pvA(j-lo)] so adjacent queue entries
        co-stream (microbenched: interleaved 4.62 Gcol/s, grouped 4.67,
        current-style serial PV 2.33).
  - exp split per batch across ACT (exact, 1024-wide) and DVE (one-op
    Schraudolph fast-exp: bf16_bits(e^x) ~= int16(EXPA*x+EXPB)), with a
    cfg-tunable per-batch assignment (A=all-ACT, D=all-DVE, S=split).
  - Tail per i-tile (spread over batches 1/2/5/10/13 of the next i-tile,
    emitted BEFORE that batch's exp so the ops sit at the head of their
    engine queues): merge -> lrow (denominator row pulled to partition 0
    via ACT Identity; DVE cannot shift partitions) -> 1/l on DVE ->
    broadcast via a K=1 ones-matmul into a stage psum bank -> ag = mg *
    bcast on DVE (f16) -> z = W4^T @ ag as a K=64 (0,0) half-tile (it
    co-streams with PV halves) -> yt = (z + b4) + x in one DVE STT, fp16
    out (y stored fp16: 2MB/core of write wire instead of 4MB).
  - PSUM: stage 3x[128,1024] (6 banks) + pvA + pvB.  The deep stage
    rotation decouples the PE from exp latency (S of batch t+3 waits only
    on the exp of batch t).  All scratch (conv psum, z, bcast) lives in
    the stage rotation; warm-up/fillers write the pv banks only in
    windows with no live PV chain.
  - Phase A (conv1x1 theta/phi/gT): emitted lazily chunk by chunk chasing
    the x DMA stream (x chunk-major contiguous in DRAM, 512-col sub-DMAs
    ordered by first use over the three DGE queues); theta/phi psum evac
    on ACT (Identity+bias), gT on DVE.  memset-fed warm-up matmuls
    un-gate the PE HAM clock before the first x chunk lands.
"""

import sys
from contextlib import ExitStack

import numpy as np

if "/opt/trn_rl_repo" not in sys.path:
    sys.path.insert(0, "/opt/trn_rl_repo")

C = 256
C4 = 64
B = 8
H = W = 64
N = H * W          # 4096
NI = 512           # i-tile width (matmul free dim)
NJ = 128           # j-block (S^T partition dim)
N_ITILES = N // NI   # 8
N_JBLKS = N // NJ    # 32

_CACHE = {}


def _build(cfg):
    import concourse.tile as tile
    from concourse import bacc, mybir

    F32 = mybir.dt.float32
    F16 = mybir.dt.float16

    nc = bacc.Bacc("TRN2", target_bir_lowering=False, debug=False,
                   num_devices=B)

    aps = dict(
        # x chunk-major: 8 chunks of [128, 1024], each a fully-contiguous
        # 256KB block so the input DMAs stream at full rate.
        x_d=nc.dram_tensor("x", [8 * 128, 1024], F16,
                           kind="ExternalInput").ap(),
        w1_d=nc.dram_tensor("w1t", [128, 256], F16, kind="ExternalInput").ap(),
        w2_d=nc.dram_tensor("w2t", [128, 256], F16, kind="ExternalInput").ap(),
        w3_d=nc.dram_tensor("w3t", [128, 128], F16, kind="ExternalInput").ap(),
        w4_d=nc.dram_tensor("w4t", [C4, C], F16,
                            kind="ExternalInput").ap(),
        b4c_d=nc.dram_tensor("b4c", [128, 2], F32,
                             kind="ExternalInput").ap(),
        b123_d=nc.dram_tensor("b123", [128, 3], F32, kind="ExternalInput").ap(),
        b3bc_d=nc.dram_tensor("b3bc", [128, C4], F32, kind="ExternalInput").ap(),
        rsc_d=nc.dram_tensor("rscratch", [2, NI], F32, kind="Internal").ap(),
        y_d=nc.dram_tensor("y", [C, N], F16, kind="ExternalOutput").ap(),
    )
    if cfg.get("dbg"):
        aps["mgdbg_d"] = nc.dram_tensor(
            "mgdbg", [C4 + 1, N], F32, kind="ExternalOutput").ap()
        aps["agdbg_d"] = nc.dram_tensor(
            "agdbg", [C4 + 1, N], F16, kind="ExternalOutput").ap()

    with tile.TileContext(nc) as tc:
        _body(nc, tc, cfg, aps)
    nc.compile()
    return nc


def _body(nc, tc, cfg, aps):
    from concourse import bass as cbass
    from concourse import mybir
    from concourse.alu_op_type import AluOpType as Alu

    F32 = mybir.dt.float32
    F16 = mybir.dt.float16
    BF16 = mybir.dt.bfloat16
    I16 = mybir.dt.int16
    Exp = mybir.ActivationFunctionType.Exp
    Ident = mybir.ActivationFunctionType.Identity

    # Schraudolph fast-exp constants: bf16_bits(e^x) ~= int16(EXPA*x + EXPB)
    EXPA = 184.6650085170266          # 2^7 / ln 2
    EXPB = 16256.5 - cfg.get("expc", 4.7)   # 127*128 + trunc-comp - center
    # Per-batch exp engine assignment (batch index within an i-tile):
    # one WIDE op per batch, alternating engines — cheaper in total
    # engine-time than splitting every batch (1100+1223 per 2 batches vs
    # 2x1390).  'act_full' batches run exact ACT exp, 'dve_full' the
    # Schraudolph DVE fast-exp, anything else splits 512/512.
    ACT_FULL = set(cfg.get("act_full", (1, 3, 5, 7, 9, 11, 13, 15)))
    DVE_FULL = set(cfg.get("dve_full", (0, 2, 4, 6, 8, 10, 12, 14)))
    I0_ACT = set(cfg.get("i0_act", (3, 6, 9, 12, 15)))
    I0_DVE = set(cfg.get("i0_dve", (2, 4, 7, 10, 13, 16)))

    x_d, y_d = aps["x_d"], aps["y_d"]

    with ExitStack() as st:
        sb = st.enter_context(tc.tile_pool(name="sb", bufs=1))

        # ---- static SBUF tensors ----
        xr_sb = sb.tile([128, 2 * N], F16, tag="xr_sb")
        # theta/phi duplicated across both partition halves (rows 64-127 =
        # rows 0-63) so S^T matmul pairs can row-pack the full PE array.
        th_t = [sb.tile([128, NI], F16, tag=f"th{n}", name=f"tht{n}")
                for n in range(N_ITILES)]
        ph_t = [sb.tile([128, NI], F16, tag=f"ph{n}", name=f"pht{n}")
                for n in range(N_ITILES)]
        gt_t = [sb.tile([128, 4 * (C4 + 1)], BF16, tag=f"gt{n}",
                        name=f"gtt{n}") for n in range(N_ITILES)]

        def ph_ap(jb):
            return ph_t[jb // 4][:, (jb % 4) * NJ:(jb % 4 + 1) * NJ]

        def gt_ap(jb):
            o = (jb % 4) * (C4 + 1)
            return gt_t[jb // 4][:, o:o + C4 + 1]

        w1_sb = sb.tile([128, 256], F16, tag="w1_sb")       # dup-M k-tiles
        w2_sb = sb.tile([128, 256], F16, tag="w2_sb")
        w3_sb = sb.tile([128, 128], F16, tag="w3_sb")
        w4_sb = sb.tile([C4, C], F16, tag="w4_sb")
        b4c_sb = sb.tile([128, 2], F32, tag="b4c_sb")
        b123_sb = sb.tile([128, 3], F32, tag="b123_sb")
        ones_sb = sb.tile([1, 128], F32, tag="ones_sb")
        ones_r_sb = sb.tile([1, C4], BF16, tag="ones_r_sb")
        b3bc_sb = sb.tile([128, C4], F32, tag="b3bc_sb")

        # ---- PSUM pools (8 banks: stage 3x2 + pvA 1 + pvB 1).  The deep
        # stage rotation (3 bufs) decouples the PE from exp latency: the
        # S-pair of batch t+3 only waits for the exp of batch t.  All
        # scratch (phase-A conv psum, z tiles, final bcast) lives in the
        # stage rotation; the warm-up burst and fillers write the pv banks
        # in windows where no PV chain is live.
        HALF = 1024                     # cols per staging half (2 banks)
        JPB = HALF // NI                # j-blocks per exp batch
        ps_stage = st.enter_context(
            tc.tile_pool(name="ps_stage", bufs=3, space="PSUM"))
        ps_pva = st.enter_context(
            tc.tile_pool(name="ps_pva", bufs=1, space="PSUM"))
        ps_pvb = st.enter_context(
            tc.tile_pool(name="ps_pvb", bufs=1, space="PSUM"))
        pt_pool = st.enter_context(tc.tile_pool(name="pt", bufs=6))
        dv_pool = st.enter_context(tc.tile_pool(name="dv", bufs=3))
        mg_pool = st.enter_context(tc.tile_pool(name="mg", bufs=2))
        y_pool = st.enter_context(tc.tile_pool(name="yp", bufs=4))

        # ---- PE warm-up burst: memset-fed bf16 matmuls with no DMA deps,
        # queued first so the HAM clock gate un-throttles (1.2->2.4GHz)
        # while the x DMA is still in flight.
        warm_w = sb.tile([128, 128], BF16, tag="warm_w")
        warm_x = sb.tile([128, 256], BF16, tag="warm_x")
        nc.vector.memset(warm_w[:], 0.5)
        nc.vector.memset(warm_x[:], 0.5)
        # the warm burst writes the (not-yet-used) pvA bank; the first real
        # PV chain matmul is start=True so the garbage never leaks.
        warm_ps = ps_pva.tile([128, 256], F32, tag="pva", name="warmps")
        for r in range(cfg.get("warm", 12)):
            nc.tensor.matmul(warm_ps[:], warm_w[:], warm_x[:],
                             start=True, stop=True)

        def filler(tile_):
            # one always-ready matmul: keeps the HAM activity monitor fed
            # through dependency stalls (a >~3.4us PE-idle window throttles
            # the PE clock to 1.2GHz for the next several us).  Caller must
            # pass a psum region with no live data.
            nc.tensor.matmul(tile_[:, 0:256], warm_w[:], warm_x[:],
                             start=True, stop=True)

        # Input DMAs are spread across the three DMA-capable queues (SP
        # hardware DGE, gpsimd software DGE, ACT hardware DGE) and ordered
        # x-chunk-0 FIRST: the real HBM transfer of x takes ~6-8us, so the
        # first conv chunk's data must be at the head of the transfer
        # stream, with the (small, fast) weight transfers interleaved after.
        def x_dma(eng, p, fine=False):
            # quarter p covers sbuf col ranges [p*2NI, (p+1)*2NI) in both
            # k-halves = dram chunks k*4+p, each a [128,1024] contiguous
            # 256KB block.  The head quarter (fine=True) is split into
            # 512-col sub-DMAs ordered (k0,c0),(k1,c0),(k0,c1),(k1,c1) so
            # the first conv chunk's contraction data arrives in half the
            # time; the bulk quarters use full 2KB partition lines.
            if fine:
                for c in range(2):
                    for k in range(2):
                        c0 = k * N + p * 2 * NI + c * NI
                        ch = k * 4 + p
                        eng.dma_start(
                            xr_sb[:, c0:c0 + NI],
                            x_d[ch * 128:(ch + 1) * 128,
                                c * NI:(c + 1) * NI])
                return
            for k in range(2):
                c0 = k * N + p * 2 * NI
                ch = k * 4 + p
                eng.dma_start(xr_sb[:, c0:c0 + 2 * NI],
                              x_d[ch * 128:(ch + 1) * 128, :])

        # x arrival order must match conv-chunk consumption order: p0 on
        # the SP queue, p1/p2 in parallel on the gpsimd/ACT queues, p3
        # queued behind p0 on SP (consumed last, arrives last).
        nc.sync.dma_start(w1_sb[:], aps["w1_d"][:])
        nc.sync.dma_start(w2_sb[:], aps["w2_d"][:])
        nc.sync.dma_start(b123_sb[:], aps["b123_d"][:])
        x_dma(nc.sync, 0, fine=True)
        # gate the other two queues behind the critical head of the wire
        # (w1/w2/b123 + x chunk 0): the HBM wire is shared, so without the
        # gate their x transfers steal bandwidth and the first conv's data
        # arrives ~7us late.
        gate_sb = sb.tile([128, 3], F32, tag="gate_sb")
        nc.gpsimd.dma_start(gate_sb[:], b123_sb[:])
        nc.gpsimd.dma_start(w3_sb[:], aps["w3_d"][:])
        nc.gpsimd.dma_start(b3bc_sb[:], aps["b3bc_d"][:])
        x_dma(nc.gpsimd, 1)
        nc.scalar.dma_start(w4_sb[:], aps["w4_d"][:])
        nc.scalar.dma_start(b4c_sb[:], aps["b4c_d"][:])
        gate2_sb = sb.tile([128, 3], F32, tag="gate2_sb")
        nc.scalar.dma_start(gate2_sb[:], b123_sb[:])
        x_dma(nc.scalar, 2)
        x_dma(nc.sync, 3)
        nc.vector.memset(ones_sb[:], 1.0)
        nc.vector.tensor_copy(ones_r_sb[:], ones_sb[:, 0:C4])
        ones128_sb = sb.tile([128, C4], BF16, tag="ones128_sb")
        nc.vector.memset(ones128_sb[:], 1.0)
        ones_col = sb.tile([128, N_JBLKS], F32, tag="ones_col")
        nc.vector.memset(ones_col[:], 1.0)
        for n in range(N_ITILES):
            nc.vector.tensor_copy(
                gt_t[n][:].rearrange("p (j c) -> p j c", c=C4 + 1)
                [:, :, C4:C4 + 1],
                ones_col[:, 4 * n:4 * n + 4]
                .rearrange("p (j c) -> p j c", c=1))
        ag_t = [sb.tile([C4, NI], F16, tag=f"ag{p}", name=f"agt{p}")
                for p in range(2)]

        # ---- phase A: conv1x1 chunks, emitted lazily (interleaved with
        # i-tile 0 of the main loop so each engine queue stays in
        # x-DMA-chunk dependency order).
        def emit_chunk(n):
            # theta/phi conv psum rotates through the misc bank, decoupled
            # from the S staging rotation.  PSUM evac on ACT (Identity +
            # per-partition bias — same table set as Exp).
            for dst_t, w_sb_, col in ((ph_t, w2_sb, 1), (th_t, w1_sb, 0)):
                ps = ps_stage.tile([128, NI], F32, tag="stage",
                                   name=f"cps{n}_{col}")
                for k in range(2):
                    nc.tensor.matmul(
                        ps[:], w_sb_[:, k * 128:(k + 1) * 128],
                        xr_sb[:, k * N + n * NI:k * N + (n + 1) * NI],
                        start=(k == 0), stop=(k == 1))
                nc.scalar.activation(dst_t[n][:], ps[:], Ident,
                                     bias=b123_sb[:, col:col + 1])
            # gT direct: 4 j-blocks merged into one psum tile; evac on DVE
            tp = ps_stage.tile([128, 4 * C4], F32, tag="stage",
                               name=f"gps{n}")
            for q in range(4):
                nb = 4 * n + q
                for k in range(2):
                    nc.tensor.matmul(
                        tp[:, q * C4:(q + 1) * C4],
                        xr_sb[:, k * N + nb * NJ:k * N + (nb + 1) * NJ],
                        w3_sb[:, k * C4:(k + 1) * C4],
                        start=(k == 0), stop=(k == 1))
            for q in range(4):
                o = (4 * n + q) % 4 * (C4 + 1)
                nc.vector.scalar_tensor_tensor(
                    gt_t[n][:, o:o + C4], tp[:, q * C4:(q + 1) * C4], 1.0,
                    b3bc_sb[:], Alu.mult, Alu.add)

        chunks_done = [0]

        def ensure_chunks(upto):
            while chunks_done[0] <= upto:
                emit_chunk(chunks_done[0])
                chunks_done[0] += 1

        # ---- main loop ----
        def make_batches(i):
            # i == 0: start with single-j-block batches so the first exps
            # arrive quickly while phase A still chases the x DMA.
            sizes = [1, 1] if i == 0 else []
            done = sum(sizes)
            while done < N_JBLKS:
                nb = min(JPB, N_JBLKS - done)
                sizes.append(nb)
                done += nb
            out, j = [], 0
            for s in sizes:
                out.append(list(range(j, j + s)))
                j += s
            return out

        pvs = [None] * N_ITILES   # (pvA_tile, pvB_tile)
        mgs = [None] * N_ITILES   # merged SBUF tile

        def pv_half(i, jb, pt, k, hi):
            # one PV j-half: hi=False -> rows 0-63 into pvA, hi=True ->
            # rows 64-127 into pvB.  The appended ones column makes row 64
            # of each chain the partial softmax denominators.
            pva, pvb = pvs[i]
            lo = 64 if hi else 0
            dst = pvb if hi else pva
            nc.tensor.matmul(
                dst[0:C4 + 1, :],
                gt_ap(jb)[lo:lo + 64, :],
                pt[lo:lo + 64, k * NI:(k + 1) * NI],
                start=(jb == 0), stop=(jb == N_JBLKS - 1),
                tile_position=(lo, 0))

        def make_halves(i, b, pt, batches):
            out = []
            for k, jb in enumerate(batches[b]):
                # B-half (rows 64-127) first so it follows an even-row S
                # tile; then the A-half.  At the chain RESTART (first jbs
                # of an i-tile) the A-half goes first: it only waits on
                # the merge's ACT read of the pvA bank, not the later DVE
                # add that frees pvB.
                if jb < 4:
                    out.append((i, jb, pt, k, False))
                    out.append((i, jb, pt, k, True))
                else:
                    out.append((i, jb, pt, k, True))
                    out.append((i, jb, pt, k, False))
            return out

        def emit_s(i, b, batches, halves):
            if i == 0:
                ensure_chunks(max(batches[b]) // 4)
            # row-packed pairs: even j-blocks on PE rows 0-63, odd on
            # 64-127 (theta/phi are duplicated across halves), interleaved
            # with pending PV halves on the opposite row group.
            if i == 0 and b == 0:
                stage_t = ps_stage.tile([128, NI], F32, tag="stage",
                                        name="stage00")
            else:
                stage_t = ps_stage.tile([128, HALF], F32, tag="stage",
                                        name=f"stage_{i}_{b}")
            half = stage_t[:, 0:len(batches[b]) * NI]
            for k, jb in enumerate(batches[b]):
                lo = (jb % 2) * C4
                nc.tensor.matmul(
                    half[:, k * NI:(k + 1) * NI],
                    ph_ap(jb)[lo:lo + C4, :],
                    th_t[i][lo:lo + C4, :],
                    start=True, stop=True,
                    tile_position=(lo, 0))
                if halves:
                    pv_half(*halves.pop(0))
            return half

        def emit_exp(i, b, half, batches):
            blist = batches[b]
            w = len(blist) * NI
            pt = pt_pool.tile([128, HALF], BF16, tag="pt")
            if len(blist) == 1:
                # i0 pipeline-fill singles: DVE (ACT is busy with conv evacs)
                nc.vector.tensor_scalar(
                    pt[:, 0:NI].bitcast(I16), half[:, 0:NI],
                    EXPA, EXPB, Alu.mult, Alu.add)
            elif b in (ACT_FULL if i > 0 else I0_ACT):
                nc.scalar.activation(pt[:, 0:w], half[:], Exp)
            elif b in (DVE_FULL if i > 0 else I0_DVE):
                nc.vector.tensor_scalar(
                    pt[:, 0:w].bitcast(I16), half[:],
                    EXPA, EXPB, Alu.mult, Alu.add)
            else:
                # split the batch across both engines: exact exp for jb0 on
                # ACT, Schraudolph for jb1 on DVE — concurrently
                nc.scalar.activation(pt[:, 0:NI], half[:, 0:NI], Exp)
                nc.vector.tensor_scalar(
                    pt[:, NI:2 * NI].bitcast(I16), half[:, NI:2 * NI],
                    EXPA, EXPB, Alu.mult, Alu.add)
            return pt

        def emit_merge(i):
            # mg = pvA + pvB (channel rows of both psum chains) -> SBUF f32
            # (ACT evacuates chain A, DVE adds chain B on top), and the two
            # partial denominator rows are pulled down to partition 0 via
            # ACT Identity (ACT supports the partition shift; DVE does
            # not).  All pv-bank reads happen here, freeing the banks for
            # the next i-tile's chains.
            pva, pvb = pvs[i]
            mg = mg_pool.tile([C4 + 1, NI], F32, tag="mg", name=f"mg{i}")
            nc.scalar.activation(mg[:], pva[0:C4 + 1, :], Ident)
            nc.vector.tensor_tensor(mg[:], pvb[0:C4 + 1, :], mg[:], Alu.add)
            mgs[i] = mg
            if cfg.get("dbg"):
                nc.sync.dma_start(
                    aps["mgdbg_d"][:, i * NI:(i + 1) * NI], mg[:])
            return mg

        tail_ag = {}

        def tail_recip(i, dram_bounce=True):
            # 1/l then broadcast [1,NI] -> [C4,NI].  Mid-loop: a gpsimd-DMA
            # DRAM bounce (frees PE+DVE; its ~3us latency hides under the
            # following batches).  Final tail: a K=1 ones-matmul (latency
            # critical).
            # row 64 of mg (the merged denominator row) is pulled down to
            # partition 0 via ACT Identity (ACT supports the partition
            # shift; DVE ops do not), then 1/l on DVE at base 0, broadcast
            # [1,NI] -> [C4,NI] via a K=1 ones-matmul into a stage-psum
            # bank (the old gpsimd DRAM bounce had a multi-us round trip
            # that stalled the z matmuls at every i-tile boundary).
            mg = mgs[i]
            lrow = dv_pool.tile([1, NI], F32, tag="lrow")
            nc.scalar.activation(lrow[:], mg[C4:C4 + 1, :], Ident)
            recip = dv_pool.tile([1, NI], F32, tag="recip")
            nc.vector.reciprocal_approx_fast(recip[:], lrow[:])
            recip_r = dv_pool.tile([1, NI], BF16, tag="recip_r")
            nc.vector.tensor_copy(recip_r[:], recip[:])
            bc = ps_stage.tile([128, NI], F32, tag="stage", name=f"bc{i}")
            nc.tensor.matmul(bc[0:C4, :], ones_r_sb[:], recip_r[:],
                             start=True, stop=True)
            return bc[0:C4, :]   # ag TT reads the psum directly (DVE)

        def tail_ag_mul(i, bcast, fast=False):
            # ag65 rows 0-63 = mg * (1/l); row 64 is preset ones (-> +b4
            # via w4_sb row 64 in the z matmul).  DVE (reads the bcast
            # psum bank directly).
            mg = mgs[i]
            ag = ag_t[i % 2]
            nc.vector.tensor_tensor(ag[0:C4, :], mg[0:C4, :], bcast[:],
                                    Alu.mult)
            if cfg.get("dbg"):
                nc.sync.dma_start(
                    aps["agdbg_d"][:, i * NI:(i + 1) * NI], ag[:])
            return ag

        def tail_z(i, ag, h, split=False):
            # z = W4^T @ ag as a K=64 row-half (0,0) tile — co-streams with
            # neighboring h64 PV halves instead of serializing the PE.
            # yt = (z + b4) + x in one DVE STT, fp16 out.
            z = ps_stage.tile([128, NI], F32, tag="stage",
                              name=f"z{i}_{h}")
            nc.tensor.matmul(z[:], w4_sb[0:C4, h * 128:(h + 1) * 128],
                             ag[:], start=True, stop=True,
                             tile_position=(0, 0))
            yt = y_pool.tile([128, NI], F16, tag="yt")
            if split:
                # final tail only: halve the yt chunks and fan the four y
                # DMAs over three queues — the kernel's very tail waits on
                # this wire.
                engs = (nc.sync, nc.gpsimd) if h == 0 else \
                       (nc.scalar, nc.sync)
                hw = NI // 2
                for q, eng in enumerate(engs):
                    sl = slice(q * hw, (q + 1) * hw)
                    nc.vector.scalar_tensor_tensor(
                        yt[:, sl], z[:, sl], b4c_sb[:, h:h + 1],
                        xr_sb[:, h * N + i * NI + q * hw:
                              h * N + i * NI + (q + 1) * hw],
                        Alu.add, Alu.add)
                    eng.dma_start(
                        y_d[h * 128:(h + 1) * 128,
                            i * NI + q * hw:i * NI + (q + 1) * hw],
                        yt[:, sl])
                return
            nc.vector.scalar_tensor_tensor(
                yt[:], z[:], b4c_sb[:, h:h + 1],
                xr_sb[:, h * N + i * NI:h * N + (i + 1) * NI],
                Alu.add, Alu.add)
            # h0 on the SP queue, h1 on the gpsimd queue so the two chunks
            # drain in parallel.
            (nc.sync if h == 0 else nc.gpsimd).dma_start(
                y_d[h * 128:(h + 1) * 128, i * NI:(i + 1) * NI], yt[:])

        # software-pipelined emission, depth 2 on the PE stream: at step t
        # the PE sees [S(t+1) interleaved with PV(t-2..t-1) halves] — PV
        # consumes exp results that are two batches old, so it never stalls
        # on the exp engines.  The previous i-tile's merge+tail chain is
        # spread over batches 2/3/6/10/13 of the next i-tile.
        all_batches = {i: make_batches(i) for i in range(N_ITILES)}
        flat = [(i, b) for i in range(N_ITILES)
                for b in range(len(all_batches[i]))]
        halves = []          # pending PV half-matmuls (closure args)
        staged = {}
        for i in range(N_ITILES):
            pva = ps_pva.tile([128, NI], F32, tag="pva", name=f"pva{i}")
            pvb = ps_pvb.tile([128, NI], F32, tag="pvb", name=f"pvb{i}")
            pvs[i] = (pva, pvb)
        staged[flat[0]] = emit_s(*flat[0], all_batches[flat[0][0]], halves)
        pend = []
        for t, (i, b) in enumerate(flat):
            # PV lags the exp stream by 2 batches.  At b==2 of each i-tile
            # the pop is HELD (depth grows to 3) so the next i-tile's PV
            # chains — which must wait for the merge's pv-bank reads —
            # don't block the in-order PE queue; b==3 double-pops to catch
            # up.
            # PV lags the exp stream by 3 batches mid-tile (a 2-batch lag
            # left PV halves waiting on exps that retire ~1.5-2 batches
            # after emission: each wide exp is ~1.2us against a ~0.8us
            # batch rhythm).  At an i-tile's first iteration ALL remaining
            # previous-tile batches drain as a burst — guaranteed PE work
            # bridging the boundary, and the chains complete early enough
            # for the merge at b==1.
            lag = cfg.get("pvlag", 3)
            while pend and pend[0][0] < i:
                halves.extend(make_halves(*pend.pop(0)))
            npop = 2 if len(pend) > lag else (1 if len(pend) >= lag else 0)
            for _ in range(npop):
                halves.extend(make_halves(*pend.pop(0)))
            # lead with two PV halves: they depend on 2-batch-old exps, so
            # they keep the PE streaming while the upcoming S-pair waits
            # for its stage bank (exp of batch t-2 on that bank).
            for _ in range(2):
                if halves:
                    pv_half(*halves.pop(0))
            if t + 1 < len(flat):
                ni, nb_ = flat[t + 1]
                staged[flat[t + 1]] = emit_s(ni, nb_, all_batches[ni], halves)
            while halves:
                pv_half(*halves.pop(0))
            if i > 0:
                # tail ops are emitted BEFORE the exp of the current batch
                # so they sit at the head of their engine queues — the
                # whole recip->bounce->ag chain must complete before the z
                # matmuls at b==10/13.
                if b == 1:
                    # the last PV halves of i-tile i-1 drained in the
                    # boundary burst at b==0 — the merge follows them.
                    emit_merge(i - 1)
                elif b == 2:
                    tail_ag["bc"] = tail_recip(i - 1)
                elif b == 3:
                    # consume the bc psum bank asap: the stage tile
                    # allocated 3 rotations after bc would otherwise stall
                    # on this read.
                    tail_ag["ag"] = tail_ag_mul(i - 1, tail_ag.pop("bc"))
            pt = emit_exp(i, b, staged.pop((i, b)), all_batches[i])
            if i == 0 and b <= 1:
                filler(warm_ps)
                filler(warm_ps)
            pend.append((i, b, pt, all_batches[i]))
            if i > 0:
                if b == 10:
                    tail_z(i - 1, tail_ag["ag"], 0)
                elif b == 13:
                    tail_z(i - 1, tail_ag.pop("ag"), 1)
        while pend:
            halves.extend(make_halves(*pend.pop(0)))
            while halves:
                pv_half(*halves.pop(0))
        # final tail: latency-critical (nothing hides it), so broadcast via
        # a ones-matmul instead of the 2-DMA DRAM bounce, ag on DVE, with
        # warm fillers interleaved so the HAM clock gate stays hot.
        i = N_ITILES - 1
        emit_merge(i)
        fill_t = ps_pva.tile([128, 256], F32, tag="pva", name="fin_fill")
        # the serial exp->merge->recip chain ahead of the bc matmul is
        # ~3.5us; the fillers must sit BEFORE the (blocked) bc matmul in
        # the in-order PE queue to keep the HAM activity monitor fed.
        for _ in range(cfg.get("fillZ", 7)):
            filler(fill_t)
        bcast_last = tail_recip(i, dram_bounce=False)
        ag_last = tail_ag_mul(i, bcast_last, fast=True)
        filler(fill_t)
        filler(fill_t)
        tail_z(i, ag_last, 0, split=True)
        filler(fill_t)
        filler(fill_t)
        tail_z(i, ag_last, 1, split=True)


def _prepare_core_inputs(x_b, W1, b1, W2, b2, W3, b3, W4, b4):
    def ktile(wT, m):
        # [256, m] -> [128, 2*m] (two k-tiles side by side)
        return np.ascontiguousarray(
            wT.reshape(2, 128, m).transpose(1, 0, 2).reshape(128, 2 * m))

    def dup(wT):
        # duplicate output channels across both halves: [256,64] -> [256,128]
        return np.concatenate([wT, wT], axis=1)

    f16 = np.float16
    z64 = np.zeros(C4, np.float32)
    xkt = x_b.reshape(2, 128, N).transpose(1, 0, 2).reshape(128, 2 * N)
    return {
        "x": np.ascontiguousarray(
            xkt.reshape(128, 8, 1024).transpose(1, 0, 2)
            .reshape(8 * 128, 1024)).astype(f16),
        "w1t": ktile(dup(W1.T), 128).astype(f16),
        "w2t": ktile(dup(W2.T), 128).astype(f16),
        "w3t": ktile(W3.T, C4).astype(f16),
        "w4t": np.ascontiguousarray(W4.T).astype(f16),
        "b4c": np.ascontiguousarray(b4.reshape(2, 128).T.copy()
                                    .astype(np.float32)),
        "b123": np.ascontiguousarray(
            np.stack([np.r_[b1, b1], np.r_[b2, b2], np.r_[b3, z64]], axis=1)),
        "b3bc": np.ascontiguousarray(
            np.broadcast_to(b3.reshape(1, C4), (128, C4)).copy()),
    }


def kernel(x, W1, b1, W2, b2, W3, b3, W4, b4, _trace=False, _cfg=None):
    from concourse import bass_utils

    cfg = dict(_cfg or {})
    key = tuple(sorted((k, tuple(v) if isinstance(v, (list, tuple)) else v)
                       for k, v in cfg.items()))
    if key not in _CACHE:
        _CACHE[key] = _build(cfg)
    nc = _CACHE[key]

    x = np.asarray(x, dtype=np.float32)
    xf = x.reshape(B, C, N)
    args = [np.asarray(a, dtype=np.float32)
            for a in (W1, b1, W2, b2, W3, b3, W4, b4)]
    in_maps = [_prepare_core_inputs(xf[b], *args) for b in range(B)]
    res = bass_utils.run_bass_kernel_spmd(
        nc, in_maps, core_ids=list(range(B)), trace=_trace)
    out = np.stack([res.results[b]["y"].astype(np.float32).reshape(C, H, W)
                    for b in range(B)])
    if _trace:
        kernel.last_exec_time_ns = res.exec_time_ns
    return out


# revision 60
# speedup vs baseline: 1.0134x; 1.0134x over previous
"""Trainium2 Bass kernel for nn_Atten_Block (non-local attention block).

Reference computation per batch element b (C=256, C4=64, H=W=64, N=4096):
    theta = W1 @ x + b1          [C4, N]
    phi   = W2 @ x + b2          [C4, N]
    g     = W3 @ x + b3          [C4, N]
    S     = theta^T @ phi        [N, N]
    A     = softmax(S, axis=-1)
    attn_g[c,i] = sum_j g[c,j] A[i,j]
    y     = x + W4 @ attn_g + b4

Sharding: data-parallel over batch B=8 across the 8 NeuronCores (one batch
element per core).

Per-core algorithm v2 — keep the PE in sustained 2-stream mode:
  - The PE streams ONE rhs column per cycle per resident tile, and can run
    two tiles CONCURRENTLY when they sit on different row-groups (rows
    0-63 / 64-127) and drain to different PSUM banks (measured 4.6 Gcol/s
    vs 2.33 for serial full-row chains).  So EVERY hot matmul is emitted as
    a half-array tile:
      * S^T tiles (K=64) row-pack even jb on rows 0-63, odd on 64-127
        (theta/phi duplicated across partition halves), as before.
      * PV is split into j-half chains: for each 128-j block, the lower
        64 j's accumulate into psum bank pvA via a (0,0) tile and the upper
        64 j's into pvB via a (64,0) tile.  The two chains (plus their
        appended ones-column -> partial softmax denominators) are merged
        after the i-tile completes: mg = pvA + pvB via one ACT Identity
        evac + one DVE add, into SBUF.
      * Emission interleaves S and PV halves on opposite row groups
        [S_even | pvB(j-hi)] [S_odd | pvA(j-lo)] so adjacent queue entries
        co-stream (microbenched: interleaved 4.62 Gcol/s, grouped 4.67,
        current-style serial PV 2.33).
  - exp split per batch across ACT (exact, 1024-wide) and DVE (one-op
    Schraudolph fast-exp: bf16_bits(e^x) ~= int16(EXPA*x+EXPB)), with a
    cfg-tunable per-batch assignment (A=all-ACT, D=all-DVE, S=split).
  - Tail per i-tile (spread over batches 1/2/5/10/13 of the next i-tile,
    emitted BEFORE that batch's exp so the ops sit at the head of their
    engine queues): merge -> lrow (denominator row pulled to partition 0
    via ACT Identity; DVE cannot shift partitions) -> 1/l on DVE ->
    broadcast via a K=1 ones-matmul into a stage psum bank -> ag = mg *
    bcast on DVE (f16) -> z = W4^T @ ag as a K=64 (0,0) half-tile (it
    co-streams with PV halves) -> yt = (z + b4) + x in one DVE STT, fp16
    out (y stored fp16: 2MB/core of write wire instead of 4MB).
  - PSUM: stage 3x[128,1024] (6 banks) + pvA + pvB.  The deep stage
    rotation decouples the PE from exp latency (S of batch t+3 waits only
    on the exp of batch t).  All scratch (conv psum, z, bcast) lives in
    the stage rotation; warm-up/fillers write the pv banks only in
    windows with no live PV chain.
  - Phase A (conv1x1 theta/phi/gT): emitted lazily chunk by chunk chasing
    the x DMA stream (x chunk-major contiguous in DRAM, 512-col sub-DMAs
    ordered by first use over the three DGE queues); theta/phi psum evac
    on ACT (Identity+bias), gT on DVE.  memset-fed warm-up matmuls
    un-gate the PE HAM clock before the first x chunk lands.
"""

import sys
from contextlib import ExitStack

import numpy as np

if "/opt/trn_rl_repo" not in sys.path:
    sys.path.insert(0, "/opt/trn_rl_repo")

C = 256
C4 = 64
B = 8
H = W = 64
N = H * W          # 4096
NI = 512           # i-tile width (matmul free dim)
NJ = 128           # j-block (S^T partition dim)
N_ITILES = N // NI   # 8
N_JBLKS = N // NJ    # 32

_CACHE = {}


def _build(cfg):
    import concourse.tile as tile
    from concourse import bacc, mybir

    F32 = mybir.dt.float32
    F16 = mybir.dt.float16

    nc = bacc.Bacc("TRN2", target_bir_lowering=False, debug=False,
                   num_devices=B)

    aps = dict(
        # x chunk-major: 8 chunks of [128, 1024], each a fully-contiguous
        # 256KB block so the input DMAs stream at full rate.
        x_d=nc.dram_tensor("x", [8 * 128, 1024], F16,
                           kind="ExternalInput").ap(),
        w1_d=nc.dram_tensor("w1t", [128, 256], F16, kind="ExternalInput").ap(),
        w2_d=nc.dram_tensor("w2t", [128, 256], F16, kind="ExternalInput").ap(),
        w3_d=nc.dram_tensor("w3t", [128, 128], F16, kind="ExternalInput").ap(),
        w4_d=nc.dram_tensor("w4t", [C4, C], F16,
                            kind="ExternalInput").ap(),
        b4c_d=nc.dram_tensor("b4c", [128, 2], F32,
                             kind="ExternalInput").ap(),
        b123_d=nc.dram_tensor("b123", [128, 3], F32, kind="ExternalInput").ap(),
        b3bc_d=nc.dram_tensor("b3bc", [128, C4], F32, kind="ExternalInput").ap(),
        rsc_d=nc.dram_tensor("rscratch", [2, NI], F32, kind="Internal").ap(),
        y_d=nc.dram_tensor("y", [C, N], F16, kind="ExternalOutput").ap(),
    )
    if cfg.get("dbg"):
        aps["mgdbg_d"] = nc.dram_tensor(
            "mgdbg", [C4 + 1, N], F32, kind="ExternalOutput").ap()
        aps["agdbg_d"] = nc.dram_tensor(
            "agdbg", [C4 + 1, N], F16, kind="ExternalOutput").ap()

    with tile.TileContext(nc) as tc:
        _body(nc, tc, cfg, aps)
    nc.compile()
    return nc


def _body(nc, tc, cfg, aps):
    from concourse import bass as cbass
    from concourse import mybir
    from concourse.alu_op_type import AluOpType as Alu

    F32 = mybir.dt.float32
    F16 = mybir.dt.float16
    BF16 = mybir.dt.bfloat16
    I16 = mybir.dt.int16
    Exp = mybir.ActivationFunctionType.Exp
    Ident = mybir.ActivationFunctionType.Identity

    # Schraudolph fast-exp constants: bf16_bits(e^x) ~= int16(EXPA*x + EXPB)
    EXPA = 184.6650085170266          # 2^7 / ln 2
    EXPB = 16256.5 - cfg.get("expc", 4.7)   # 127*128 + trunc-comp - center
    # Per-batch exp engine assignment (batch index within an i-tile):
    # one WIDE op per batch, alternating engines — cheaper in total
    # engine-time than splitting every batch (1100+1223 per 2 batches vs
    # 2x1390).  'act_full' batches run exact ACT exp, 'dve_full' the
    # Schraudolph DVE fast-exp, anything else splits 512/512.
    ACT_FULL = set(cfg.get("act_full", (1, 3, 5, 7, 9, 11, 13, 15)))
    DVE_FULL = set(cfg.get("dve_full", (0, 2, 4, 6, 8, 10, 12, 14)))
    I0_ACT = set(cfg.get("i0_act", (3, 6, 9, 12, 15)))
    I0_DVE = set(cfg.get("i0_dve", (2, 4, 7, 10, 13, 16)))

    x_d, y_d = aps["x_d"], aps["y_d"]

    with ExitStack() as st:
        sb = st.enter_context(tc.tile_pool(name="sb", bufs=1))

        # ---- static SBUF tensors ----
        xr_sb = sb.tile([128, 2 * N], F16, tag="xr_sb")
        # theta/phi duplicated across both partition halves (rows 64-127 =
        # rows 0-63) so S^T matmul pairs can row-pack the full PE array.
        th_t = [sb.tile([128, NI], F16, tag=f"th{n}", name=f"tht{n}")
                for n in range(N_ITILES)]
        ph_t = [sb.tile([128, NI], F16, tag=f"ph{n}", name=f"pht{n}")
                for n in range(N_ITILES)]
        gt_t = [sb.tile([128, 4 * (C4 + 1)], BF16, tag=f"gt{n}",
                        name=f"gtt{n}") for n in range(N_ITILES)]

        def ph_ap(jb):
            return ph_t[jb // 4][:, (jb % 4) * NJ:(jb % 4 + 1) * NJ]

        def gt_ap(jb):
            o = (jb % 4) * (C4 + 1)
            return gt_t[jb // 4][:, o:o + C4 + 1]

        w1_sb = sb.tile([128, 256], F16, tag="w1_sb")       # dup-M k-tiles
        w2_sb = sb.tile([128, 256], F16, tag="w2_sb")
        w3_sb = sb.tile([128, 128], F16, tag="w3_sb")
        w4_sb = sb.tile([C4, C], F16, tag="w4_sb")
        b4c_sb = sb.tile([128, 2], F32, tag="b4c_sb")
        b123_sb = sb.tile([128, 3], F32, tag="b123_sb")
        ones_sb = sb.tile([1, 128], F32, tag="ones_sb")
        ones_r_sb = sb.tile([1, C4], BF16, tag="ones_r_sb")
        b3bc_sb = sb.tile([128, C4], F32, tag="b3bc_sb")

        # ---- PSUM pools (8 banks: stage 3x2 + pvA 1 + pvB 1).  The deep
        # stage rotation (3 bufs) decouples the PE from exp latency: the
        # S-pair of batch t+3 only waits for the exp of batch t.  All
        # scratch (phase-A conv psum, z tiles, final bcast) lives in the
        # stage rotation; the warm-up burst and fillers write the pv banks
        # in windows where no PV chain is live.
        HALF = 1024                     # cols per staging half (2 banks)
        JPB = HALF // NI                # j-blocks per exp batch
        ps_stage = st.enter_context(
            tc.tile_pool(name="ps_stage", bufs=3, space="PSUM"))
        ps_pva = st.enter_context(
            tc.tile_pool(name="ps_pva", bufs=1, space="PSUM"))
        ps_pvb = st.enter_context(
            tc.tile_pool(name="ps_pvb", bufs=1, space="PSUM"))
        pt_pool = st.enter_context(tc.tile_pool(name="pt", bufs=5))
        dv_pool = st.enter_context(tc.tile_pool(name="dv", bufs=3))
        mg_pool = st.enter_context(tc.tile_pool(name="mg", bufs=2))
        y_pool = st.enter_context(tc.tile_pool(name="yp", bufs=4))

        # ---- PE warm-up burst: memset-fed bf16 matmuls with no DMA deps,
        # queued first so the HAM clock gate un-throttles (1.2->2.4GHz)
        # while the x DMA is still in flight.
        warm_w = sb.tile([128, 128], BF16, tag="warm_w")
        warm_x = sb.tile([128, 256], BF16, tag="warm_x")
        nc.vector.memset(warm_w[:], 0.5)
        nc.vector.memset(warm_x[:], 0.5)
        # the warm burst writes the (not-yet-used) pvA bank; the first real
        # PV chain matmul is start=True so the garbage never leaks.
        warm_ps = ps_pva.tile([128, 256], F32, tag="pva", name="warmps")
        for r in range(cfg.get("warm", 12)):
            nc.tensor.matmul(warm_ps[:], warm_w[:], warm_x[:],
                             start=True, stop=True)

        def filler(tile_):
            # one always-ready matmul: keeps the HAM activity monitor fed
            # through dependency stalls (a >~3.4us PE-idle window throttles
            # the PE clock to 1.2GHz for the next several us).  Caller must
            # pass a psum region with no live data.
            nc.tensor.matmul(tile_[:, 0:256], warm_w[:], warm_x[:],
                             start=True, stop=True)

        # Input DMAs are spread across the three DMA-capable queues (SP
        # hardware DGE, gpsimd software DGE, ACT hardware DGE) and ordered
        # x-chunk-0 FIRST: the real HBM transfer of x takes ~6-8us, so the
        # first conv chunk's data must be at the head of the transfer
        # stream, with the (small, fast) weight transfers interleaved after.
        def x_dma(eng, p, fine=False):
            # quarter p covers sbuf col ranges [p*2NI, (p+1)*2NI) in both
            # k-halves = dram chunks k*4+p, each a [128,1024] contiguous
            # 256KB block.  The head quarter (fine=True) is split into
            # 512-col sub-DMAs ordered (k0,c0),(k1,c0),(k0,c1),(k1,c1) so
            # the first conv chunk's contraction data arrives in half the
            # time; the bulk quarters use full 2KB partition lines.
            if fine:
                for c in range(2):
                    for k in range(2):
                        c0 = k * N + p * 2 * NI + c * NI
                        ch = k * 4 + p
                        eng.dma_start(
                            xr_sb[:, c0:c0 + NI],
                            x_d[ch * 128:(ch + 1) * 128,
                                c * NI:(c + 1) * NI])
                return
            for k in range(2):
                c0 = k * N + p * 2 * NI
                ch = k * 4 + p
                eng.dma_start(xr_sb[:, c0:c0 + 2 * NI],
                              x_d[ch * 128:(ch + 1) * 128, :])

        # x arrival order must match conv-chunk consumption order: p0 on
        # the SP queue, p1/p2 in parallel on the gpsimd/ACT queues, p3
        # queued behind p0 on SP (consumed last, arrives last).
        nc.sync.dma_start(w1_sb[:], aps["w1_d"][:])
        nc.sync.dma_start(w2_sb[:], aps["w2_d"][:])
        nc.sync.dma_start(b123_sb[:], aps["b123_d"][:])
        x_dma(nc.sync, 0, fine=True)
        # gate the other two queues behind the critical head of the wire
        # (w1/w2/b123 + x chunk 0): the HBM wire is shared, so without the
        # gate their x transfers steal bandwidth and the first conv's data
        # arrives ~7us late.
        gate_sb = sb.tile([128, 3], F32, tag="gate_sb")
        nc.gpsimd.dma_start(gate_sb[:], b123_sb[:])
        nc.gpsimd.dma_start(w3_sb[:], aps["w3_d"][:])
        nc.gpsimd.dma_start(b3bc_sb[:], aps["b3bc_d"][:])
        x_dma(nc.gpsimd, 1)
        nc.scalar.dma_start(w4_sb[:], aps["w4_d"][:])
        nc.scalar.dma_start(b4c_sb[:], aps["b4c_d"][:])
        gate2_sb = sb.tile([128, 3], F32, tag="gate2_sb")
        nc.scalar.dma_start(gate2_sb[:], b123_sb[:])
        x_dma(nc.scalar, 2)
        x_dma(nc.sync, 3)
        nc.vector.memset(ones_sb[:], 1.0)
        nc.vector.tensor_copy(ones_r_sb[:], ones_sb[:, 0:C4])
        ones128_sb = sb.tile([128, C4], BF16, tag="ones128_sb")
        nc.vector.memset(ones128_sb[:], 1.0)
        ones_col = sb.tile([128, N_JBLKS], F32, tag="ones_col")
        nc.vector.memset(ones_col[:], 1.0)
        for n in range(N_ITILES):
            nc.vector.tensor_copy(
                gt_t[n][:].rearrange("p (j c) -> p j c", c=C4 + 1)
                [:, :, C4:C4 + 1],
                ones_col[:, 4 * n:4 * n + 4]
                .rearrange("p (j c) -> p j c", c=1))
        ag_t = [sb.tile([C4, NI], F16, tag=f"ag{p}", name=f"agt{p}")
                for p in range(2)]

        # ---- phase A: conv1x1 chunks, emitted lazily (interleaved with
        # i-tile 0 of the main loop so each engine queue stays in
        # x-DMA-chunk dependency order).
        def emit_chunk(n):
            # theta/phi conv psum rotates through the misc bank, decoupled
            # from the S staging rotation.  PSUM evac on ACT (Identity +
            # per-partition bias — same table set as Exp).
            for dst_t, w_sb_, col in ((ph_t, w2_sb, 1), (th_t, w1_sb, 0)):
                ps = ps_stage.tile([128, NI], F32, tag="stage",
                                   name=f"cps{n}_{col}")
                for k in range(2):
                    nc.tensor.matmul(
                        ps[:], w_sb_[:, k * 128:(k + 1) * 128],
                        xr_sb[:, k * N + n * NI:k * N + (n + 1) * NI],
                        start=(k == 0), stop=(k == 1))
                nc.scalar.activation(dst_t[n][:], ps[:], Ident,
                                     bias=b123_sb[:, col:col + 1])
            # gT direct: 4 j-blocks merged into one psum tile; evac on DVE
            tp = ps_stage.tile([128, 4 * C4], F32, tag="stage",
                               name=f"gps{n}")
            for q in range(4):
                nb = 4 * n + q
                for k in range(2):
                    nc.tensor.matmul(
                        tp[:, q * C4:(q + 1) * C4],
                        xr_sb[:, k * N + nb * NJ:k * N + (nb + 1) * NJ],
                        w3_sb[:, k * C4:(k + 1) * C4],
                        start=(k == 0), stop=(k == 1))
            for q in range(4):
                o = (4 * n + q) % 4 * (C4 + 1)
                nc.vector.scalar_tensor_tensor(
                    gt_t[n][:, o:o + C4], tp[:, q * C4:(q + 1) * C4], 1.0,
                    b3bc_sb[:], Alu.mult, Alu.add)

        chunks_done = [0]

        def ensure_chunks(upto):
            while chunks_done[0] <= upto:
                emit_chunk(chunks_done[0])
                chunks_done[0] += 1

        # ---- main loop ----
        def make_batches(i):
            # i == 0: start with single-j-block batches so the first exps
            # arrive quickly while phase A still chases the x DMA.
            sizes = [1, 1] if i == 0 else []
            done = sum(sizes)
            while done < N_JBLKS:
                nb = min(JPB, N_JBLKS - done)
                sizes.append(nb)
                done += nb
            out, j = [], 0
            for s in sizes:
                out.append(list(range(j, j + s)))
                j += s
            return out

        pvs = [None] * N_ITILES   # (pvA_tile, pvB_tile)
        mgs = [None] * N_ITILES   # merged SBUF tile

        def pv_half(i, jb, pt, k, hi):
            # one PV j-half: hi=False -> rows 0-63 into pvA, hi=True ->
            # rows 64-127 into pvB.  The appended ones column makes row 64
            # of each chain the partial softmax denominators.
            pva, pvb = pvs[i]
            lo = 64 if hi else 0
            dst = pvb if hi else pva
            nc.tensor.matmul(
                dst[0:C4 + 1, :],
                gt_ap(jb)[lo:lo + 64, :],
                pt[lo:lo + 64, k * NI:(k + 1) * NI],
                start=(jb == 0), stop=(jb == N_JBLKS - 1),
                tile_position=(lo, 0))

        def make_halves(i, b, pt, batches):
            out = []
            for k, jb in enumerate(batches[b]):
                # B-half (rows 64-127) first so it follows an even-row S
                # tile; then the A-half.  At the chain RESTART (first jbs
                # of an i-tile) the A-half goes first: it only waits on
                # the merge's ACT read of the pvA bank, not the later DVE
                # add that frees pvB.
                if jb < 4:
                    out.append((i, jb, pt, k, False))
                    out.append((i, jb, pt, k, True))
                else:
                    out.append((i, jb, pt, k, True))
                    out.append((i, jb, pt, k, False))
            return out

        def emit_s(i, b, batches, halves):
            if i == 0:
                ensure_chunks(max(batches[b]) // 4)
            # row-packed pairs: even j-blocks on PE rows 0-63, odd on
            # 64-127 (theta/phi are duplicated across halves), interleaved
            # with pending PV halves on the opposite row group.
            if i == 0 and b == 0:
                stage_t = ps_stage.tile([128, NI], F32, tag="stage",
                                        name="stage00")
            else:
                stage_t = ps_stage.tile([128, HALF], F32, tag="stage",
                                        name=f"stage_{i}_{b}")
            half = stage_t[:, 0:len(batches[b]) * NI]
            for k, jb in enumerate(batches[b]):
                lo = (jb % 2) * C4
                nc.tensor.matmul(
                    half[:, k * NI:(k + 1) * NI],
                    ph_ap(jb)[lo:lo + C4, :],
                    th_t[i][lo:lo + C4, :],
                    start=True, stop=True,
                    tile_position=(lo, 0))
                if halves:
                    pv_half(*halves.pop(0))
            return half

        def emit_exp(i, b, half, batches):
            blist = batches[b]
            w = len(blist) * NI
            pt = pt_pool.tile([128, HALF], BF16, tag="pt")
            if len(blist) == 1:
                # i0 pipeline-fill singles: DVE (ACT is busy with conv evacs)
                nc.vector.tensor_scalar(
                    pt[:, 0:NI].bitcast(I16), half[:, 0:NI],
                    EXPA, EXPB, Alu.mult, Alu.add)
            elif b in (ACT_FULL if i > 0 else I0_ACT):
                nc.scalar.activation(pt[:, 0:w], half[:], Exp)
            elif b in (DVE_FULL if i > 0 else I0_DVE):
                nc.vector.tensor_scalar(
                    pt[:, 0:w].bitcast(I16), half[:],
                    EXPA, EXPB, Alu.mult, Alu.add)
            else:
                # split the batch across both engines: exact exp for jb0 on
                # ACT, Schraudolph for jb1 on DVE — concurrently
                nc.scalar.activation(pt[:, 0:NI], half[:, 0:NI], Exp)
                nc.vector.tensor_scalar(
                    pt[:, NI:2 * NI].bitcast(I16), half[:, NI:2 * NI],
                    EXPA, EXPB, Alu.mult, Alu.add)
            return pt

        def emit_merge(i):
            # mg = pvA + pvB (channel rows of both psum chains) -> SBUF f32
            # (ACT evacuates chain A, DVE adds chain B on top), and the two
            # partial denominator rows are pulled down to partition 0 via
            # ACT Identity (ACT supports the partition shift; DVE does
            # not).  All pv-bank reads happen here, freeing the banks for
            # the next i-tile's chains.
            pva, pvb = pvs[i]
            mg = mg_pool.tile([C4 + 1, NI], F32, tag="mg", name=f"mg{i}")
            nc.scalar.activation(mg[:], pva[0:C4 + 1, :], Ident)
            nc.vector.tensor_tensor(mg[:], pvb[0:C4 + 1, :], mg[:], Alu.add)
            mgs[i] = mg
            if cfg.get("dbg"):
                nc.sync.dma_start(
                    aps["mgdbg_d"][:, i * NI:(i + 1) * NI], mg[:])
            return mg

        tail_ag = {}

        def tail_recip(i, dram_bounce=True):
            # 1/l then broadcast [1,NI] -> [C4,NI].  Mid-loop: a gpsimd-DMA
            # DRAM bounce (frees PE+DVE; its ~3us latency hides under the
            # following batches).  Final tail: a K=1 ones-matmul (latency
            # critical).
            # row 64 of mg (the merged denominator row) is pulled down to
            # partition 0 via ACT Identity (ACT supports the partition
            # shift; DVE ops do not), then 1/l on DVE at base 0, broadcast
            # [1,NI] -> [C4,NI] via a K=1 ones-matmul into a stage-psum
            # bank (the old gpsimd DRAM bounce had a multi-us round trip
            # that stalled the z matmuls at every i-tile boundary).
            mg = mgs[i]
            lrow = dv_pool.tile([1, NI], F32, tag="lrow")
            nc.scalar.activation(lrow[:], mg[C4:C4 + 1, :], Ident)
            recip = dv_pool.tile([1, NI], F32, tag="recip")
            nc.vector.reciprocal_approx_fast(recip[:], lrow[:])
            recip_r = dv_pool.tile([1, NI], BF16, tag="recip_r")
            nc.vector.tensor_copy(recip_r[:], recip[:])
            bc = ps_stage.tile([128, NI], F32, tag="stage", name=f"bc{i}")
            nc.tensor.matmul(bc[0:C4, :], ones_r_sb[:], recip_r[:],
                             start=True, stop=True)
            return bc[0:C4, :]   # ag TT reads the psum directly (DVE)

        def tail_ag_mul(i, bcast, fast=False):
            # ag65 rows 0-63 = mg * (1/l); row 64 is preset ones (-> +b4
            # via w4_sb row 64 in the z matmul).  DVE (reads the bcast
            # psum bank directly).
            mg = mgs[i]
            ag = ag_t[i % 2]
            nc.vector.tensor_tensor(ag[0:C4, :], mg[0:C4, :], bcast[:],
                                    Alu.mult)
            if cfg.get("dbg"):
                nc.sync.dma_start(
                    aps["agdbg_d"][:, i * NI:(i + 1) * NI], ag[:])
            return ag

        def tail_z(i, ag, h, split=False):
            # z = W4^T @ ag as a K=64 row-half (0,0) tile — co-streams with
            # neighboring h64 PV halves instead of serializing the PE.
            # yt = (z + b4) + x in one DVE STT, fp16 out.
            z = ps_stage.tile([128, NI], F32, tag="stage",
                              name=f"z{i}_{h}")
            nc.tensor.matmul(z[:], w4_sb[0:C4, h * 128:(h + 1) * 128],
                             ag[:], start=True, stop=True,
                             tile_position=(0, 0))
            yt = y_pool.tile([128, NI], F16, tag="yt")
            if split:
                # final tail only: halve the yt chunks and fan the four y
                # DMAs over three queues — the kernel's very tail waits on
                # this wire.
                engs = (nc.sync, nc.gpsimd) if h == 0 else \
                       (nc.scalar, nc.sync)
                hw = NI // 2
                for q, eng in enumerate(engs):
                    sl = slice(q * hw, (q + 1) * hw)
                    nc.vector.scalar_tensor_tensor(
                        yt[:, sl], z[:, sl], b4c_sb[:, h:h + 1],
                        xr_sb[:, h * N + i * NI + q * hw:
                              h * N + i * NI + (q + 1) * hw],
                        Alu.add, Alu.add)
                    eng.dma_start(
                        y_d[h * 128:(h + 1) * 128,
                            i * NI + q * hw:i * NI + (q + 1) * hw],
                        yt[:, sl])
                return
            nc.vector.scalar_tensor_tensor(
                yt[:], z[:], b4c_sb[:, h:h + 1],
                xr_sb[:, h * N + i * NI:h * N + (i + 1) * NI],
                Alu.add, Alu.add)
            # h0 on the SP queue, h1 on the gpsimd queue so the two chunks
            # drain in parallel.
            (nc.sync if h == 0 else nc.gpsimd).dma_start(
                y_d[h * 128:(h + 1) * 128, i * NI:(i + 1) * NI], yt[:])

        # software-pipelined emission, depth 2 on the PE stream: at step t
        # the PE sees [S(t+1) interleaved with PV(t-2..t-1) halves] — PV
        # consumes exp results that are two batches old, so it never stalls
        # on the exp engines.  The previous i-tile's merge+tail chain is
        # spread over batches 2/3/6/10/13 of the next i-tile.
        all_batches = {i: make_batches(i) for i in range(N_ITILES)}
        flat = [(i, b) for i in range(N_ITILES)
                for b in range(len(all_batches[i]))]
        halves = []          # pending PV half-matmuls (closure args)
        staged = {}
        for i in range(N_ITILES):
            pva = ps_pva.tile([128, NI], F32, tag="pva", name=f"pva{i}")
            pvb = ps_pvb.tile([128, NI], F32, tag="pvb", name=f"pvb{i}")
            pvs[i] = (pva, pvb)
        staged[flat[0]] = emit_s(*flat[0], all_batches[flat[0][0]], halves)
        pend = []
        for t, (i, b) in enumerate(flat):
            # PV lags the exp stream by 2 batches.  At b==2 of each i-tile
            # the pop is HELD (depth grows to 3) so the next i-tile's PV
            # chains — which must wait for the merge's pv-bank reads —
            # don't block the in-order PE queue; b==3 double-pops to catch
            # up.
            # PV lags the exp stream by 3 batches: a 2-batch lag left the
            # PV halves waiting on exps that retire ~1.5-2 batches after
            # emission (each wide exp is ~1.2us against a ~0.8us batch
            # rhythm).
            lag = cfg.get("pvlag", 2)
            hold = (i > 0 and b == 2 and lag == 2 and not cfg.get("nohold"))
            npop = 0 if hold else (2 if len(pend) > lag else 1)
            for _ in range(npop):
                if len(pend) >= lag:
                    halves.extend(make_halves(*pend.pop(0)))
            # lead with two PV halves: they depend on 2-batch-old exps, so
            # they keep the PE streaming while the upcoming S-pair waits
            # for its stage bank (exp of batch t-2 on that bank).
            for _ in range(2):
                if halves:
                    pv_half(*halves.pop(0))
            if t + 1 < len(flat):
                ni, nb_ = flat[t + 1]
                staged[flat[t + 1]] = emit_s(ni, nb_, all_batches[ni], halves)
            while halves:
                pv_half(*halves.pop(0))
            if i > 0:
                # tail ops are emitted BEFORE the exp of the current batch
                # so they sit at the head of their engine queues — the
                # whole recip->bounce->ag chain must complete before the z
                # matmuls at b==10/13.
                if b == lag - 1:
                    # the last PV halves of i-tile i-1 drain during this
                    # iteration, just above — the merge follows them.
                    emit_merge(i - 1)
                elif b == lag:
                    tail_ag["bc"] = tail_recip(i - 1)
                elif b == lag + 1:
                    # consume the bc psum bank asap: the stage tile
                    # allocated 3 rotations after bc would otherwise stall
                    # on this read.
                    tail_ag["ag"] = tail_ag_mul(i - 1, tail_ag.pop("bc"))
            pt = emit_exp(i, b, staged.pop((i, b)), all_batches[i])
            if i == 0 and b <= 1:
                filler(warm_ps)
                filler(warm_ps)
            pend.append((i, b, pt, all_batches[i]))
            if i > 0:
                if b == 10:
                    tail_z(i - 1, tail_ag["ag"], 0)
                elif b == 13:
                    tail_z(i - 1, tail_ag.pop("ag"), 1)
        while pend:
            halves.extend(make_halves(*pend.pop(0)))
            while halves:
                pv_half(*halves.pop(0))
        # final tail: latency-critical (nothing hides it), so broadcast via
        # a ones-matmul instead of the 2-DMA DRAM bounce, ag on DVE, with
        # warm fillers interleaved so the HAM clock gate stays hot.
        i = N_ITILES - 1
        emit_merge(i)
        fill_t = ps_pva.tile([128, 256], F32, tag="pva", name="fin_fill")
        # the serial exp->merge->recip chain ahead of the bc matmul is
        # ~3.5us; the fillers must sit BEFORE the (blocked) bc matmul in
        # the in-order PE queue to keep the HAM activity monitor fed.
        for _ in range(cfg.get("fillZ", 7)):
            filler(fill_t)
        bcast_last = tail_recip(i, dram_bounce=False)
        ag_last = tail_ag_mul(i, bcast_last, fast=True)
        filler(fill_t)
        filler(fill_t)
        tail_z(i, ag_last, 0, split=True)
        filler(fill_t)
        filler(fill_t)
        tail_z(i, ag_last, 1, split=True)


def _prepare_core_inputs(x_b, W1, b1, W2, b2, W3, b3, W4, b4):
    def ktile(wT, m):
        # [256, m] -> [128, 2*m] (two k-tiles side by side)
        return np.ascontiguousarray(
            wT.reshape(2, 128, m).transpose(1, 0, 2).reshape(128, 2 * m))

    def dup(wT):
        # duplicate output channels across both halves: [256,64] -> [256,128]
        return np.concatenate([wT, wT], axis=1)

    f16 = np.float16
    z64 = np.zeros(C4, np.float32)
    xkt = x_b.reshape(2, 128, N).transpose(1, 0, 2).reshape(128, 2 * N)
    return {
        "x": np.ascontiguousarray(
            xkt.reshape(128, 8, 1024).transpose(1, 0, 2)
            .reshape(8 * 128, 1024)).astype(f16),
        "w1t": ktile(dup(W1.T), 128).astype(f16),
        "w2t": ktile(dup(W2.T), 128).astype(f16),
        "w3t": ktile(W3.T, C4).astype(f16),
        "w4t": np.ascontiguousarray(W4.T).astype(f16),
        "b4c": np.ascontiguousarray(b4.reshape(2, 128).T.copy()
                                    .astype(np.float32)),
        "b123": np.ascontiguousarray(
            np.stack([np.r_[b1, b1], np.r_[b2, b2], np.r_[b3, z64]], axis=1)),
        "b3bc": np.ascontiguousarray(
            np.broadcast_to(b3.reshape(1, C4), (128, C4)).copy()),
    }


def kernel(x, W1, b1, W2, b2, W3, b3, W4, b4, _trace=False, _cfg=None):
    from concourse import bass_utils

    cfg = dict(_cfg or {})
    key = tuple(sorted((k, tuple(v) if isinstance(v, (list, tuple)) else v)
                       for k, v in cfg.items()))
    if key not in _CACHE:
        _CACHE[key] = _build(cfg)
    nc = _CACHE[key]

    x = np.asarray(x, dtype=np.float32)
    xf = x.reshape(B, C, N)
    args = [np.asarray(a, dtype=np.float32)
            for a in (W1, b1, W2, b2, W3, b3, W4, b4)]
    in_maps = [_prepare_core_inputs(xf[b], *args) for b in range(B)]
    res = bass_utils.run_bass_kernel_spmd(
        nc, in_maps, core_ids=list(range(B)), trace=_trace)
    out = np.stack([res.results[b]["y"].astype(np.float32).reshape(C, H, W)
                    for b in range(B)])
    if _trace:
        kernel.last_exec_time_ns = res.exec_time_ns
    return out
